# revision 5
# baseline (speedup 1.0000x reference)
"""LoRA-MoE layer (base dense + top-2 routed rank-16 LoRA experts) on 8 TRN2 cores.

Strategy: data-parallel over tokens (8192 tokens -> 1024/core), all weights
replicated, zero collectives. Per-core fused Bass/Tile kernel, v2:

  phase A (paced by the xh DMA stream, chunk by chunk):
    lg^T[e,t]  = R^T.xh     (single-pass bf16 — top-2 flips are ~16/8192
                             tokens and contribute ~4e-3 rel err, well
                             under the 2e-2 gate)
    u^T[er,t]  = A^T.xh     (bf16)
  phase A2: transpose lg to [t,8] tiles, top-2 softmax chains on DVE/ACT,
    transpose w back, expand to [er,t] via one-hot matmul, us = u*Wb*2.
  phase B (ob outer, ti inner): out[t,o] = xh^T.T @ W^T accumulated over
    16 k-chunks + us^T.T @ Bc into the same PSUM tile, then staged copy +
    DMA out.  W streams as 4 o-block-major 2 MiB DMAs timed to land just
    as phase B starts; out DMAs spread evenly so the tail is one tile.
"""

import os
import sys

import numpy as np


def _ensure_concourse():
    try:
        import concourse  # noqa: F401
    except ImportError:
        for p in ("/opt/trn_rl_repo", os.path.expanduser("~/.axon_site/_ro/trn_rl_repo")):
            if os.path.isdir(p):
                sys.path.insert(0, p)
                break


_ensure_concourse()

import ml_dtypes  # noqa: E402
import concourse.bass as bass  # noqa: E402,F401
import concourse.tile as tile  # noqa: E402
from concourse import bacc, mybir  # noqa: E402

F32 = mybir.dt.float32
BF16 = mybir.dt.bfloat16
X_AX = mybir.AxisListType.X
ALU = mybir.AluOpType
ACT = mybir.ActivationFunctionType

N_CORES = 8
N_TOK = 8192          # total tokens (4 x 2048)
NT = N_TOK // N_CORES  # tokens per core = 1024
D = 2048
O = 2048
E = 8
R = 16
ER = E * R            # 128
KT = D // 128         # 16 contraction chunks
TI = NT // 128        # 8 token tiles
OBS = 4               # o blocks of 512
TBS = 2               # token blocks of 512

_NC_CACHE = {}
LAST_RESULTS = None


def _chain(nc, smallp, L):
    """Top-2 softmax weights for one [128, E] logits tile; SCALING folded."""
    m1 = smallp.tile([128, 1], F32, name="m1", tag="m1")
    nc.vector.reduce_max(m1[:], L[:], axis=X_AX)
    nm1 = smallp.tile([128, 1], F32, name="nm1", tag="nm1")
    nc.scalar.mul(nm1[:], m1[:], -1.0)
    msk = smallp.tile([128, E], F32, name="msk", tag="msk")
    nc.vector.tensor_scalar(msk[:], L[:], m1[:], -1e30, ALU.is_equal, ALU.mult)
    L2 = smallp.tile([128, E], F32, name="L2", tag="L2")
    nc.vector.tensor_tensor(L2[:], L[:], msk[:], ALU.add)
    m2 = smallp.tile([128, 1], F32, name="m2", tag="m2")
    nc.vector.reduce_max(m2[:], L2[:], axis=X_AX)
    eL = smallp.tile([128, E], F32, name="eL", tag="eL")
    nc.scalar.activation(eL[:], L[:], ACT.Exp, bias=nm1[:])
    ge = smallp.tile([128, E], F32, name="ge", tag="ge")
    nc.vector.tensor_scalar(ge[:], L[:], m2[:], None, ALU.is_ge)
    un = smallp.tile([128, E], F32, name="un", tag="un")
    nc.vector.tensor_tensor(un[:], eL[:], ge[:], ALU.mult)
    s = smallp.tile([128, 1], F32, name="s", tag="s")
    nc.vector.reduce_sum(s[:], un[:], axis=X_AX)
    r = smallp.tile([128, 1], F32, name="r", tag="r")
    nc.vector.reciprocal(r[:], s[:])
    r2 = smallp.tile([128, 1], F32, name="r2", tag="r2")
    nc.scalar.mul(r2[:], r[:], 2.0)  # fold SCALING = 2.0
    w = smallp.tile([128, E], F32, name="w", tag="w", bufs=8)
    nc.vector.tensor_scalar(w[:], un[:], r2[:], None, ALU.mult)
    return w


def _body(tc, nc, xh, WTo, AT, RT, Bc, Mm, Idn, out):
    with (
        tc.tile_pool(name="const", bufs=1) as constp,
        tc.tile_pool(name="small", bufs=4) as smallp,
        tc.tile_pool(name="stage", bufs=4) as stagep,
        tc.tile_pool(name="ps_aux", bufs=2, space="PSUM") as psauxp,
        tc.tile_pool(name="ps_u", bufs=1, space="PSUM") as psup,
        tc.tile_pool(name="ps_main", bufs=4, space="PSUM") as psmainp,
    ):
        # ---- resident SBUF tensors ----
        xh_sb = constp.tile([128, KT, NT], BF16, name="xh_sb")
        W_sb = constp.tile([128, OBS, KT, 512], BF16, name="W_sb")
        AT_sb = constp.tile([128, KT, ER], BF16, name="AT_sb")
        RT_sb = constp.tile([128, KT, E], BF16, name="RT_sb")
        Bc_sb = constp.tile([ER, O], BF16, name="Bc_sb")
        Mm_sb = constp.tile([E, ER], BF16, name="Mm_sb")
        Id_sb = constp.tile([128, 128], F32, name="Id_sb")
        lg_sb = constp.tile([E, NT], F32, name="lg_sb")
        wT_sb = constp.tile([E, NT], BF16, name="wT_sb")
        Wb_sb = constp.tile([ER, NT], F32, name="Wb_sb")
        us_sb = constp.tile([ER, NT], BF16, name="us_sb")

        # DMA order = consumption order on one HWDGE FIFO: router + A first,
        # the 16 xh chunks that pace phase A, small consts, then W in 4
        # o-block-major 2 MiB transfers that land right as phase B begins.
        nc.sync.dma_start(RT_sb[:], RT[:])
        nc.sync.dma_start(AT_sb[:], AT[:])
        for k in range(KT):
            nc.sync.dma_start(xh_sb[:, k, :], xh[k * 128:(k + 1) * 128, :])
        nc.sync.dma_start(Mm_sb[:], Mm[:])
        nc.sync.dma_start(Id_sb[:], Idn[:])
        nc.sync.dma_start(Bc_sb[:], Bc[:])
        for ob in range(OBS):
            nc.sync.dma_start(W_sb[:, ob, :, :], WTo[ob])

        # ---- phase A: router logits + LoRA-A projection, chunk-paced ----
        lg_ps = [psauxp.tile([E, 512], F32, name=f"lgps{tb}", tag="aux") for tb in range(TBS)]
        u_ps = [psup.tile([ER, 512], F32, name=f"ups{tb}", tag=f"u{tb}") for tb in range(TBS)]
        for k in range(KT):
            for tb in range(TBS):
                sl = slice(tb * 512, (tb + 1) * 512)
                nc.tensor.matmul(
                    lg_ps[tb][:], RT_sb[:, k, :], xh_sb[:, k, sl],
                    start=(k == 0), stop=(k == KT - 1),
                )
                nc.tensor.matmul(
                    u_ps[tb][:], AT_sb[:, k, :], xh_sb[:, k, sl],
                    start=(k == 0), stop=(k == KT - 1),
                )
        for tb in range(TBS):
            nc.scalar.copy(lg_sb[:, tb * 512:(tb + 1) * 512], lg_ps[tb][:])

        # ---- phase A2a: transpose logits to [t, 8] tiles + softmax chains ----
        w_tiles = []
        for ti in range(TI):
            sl = slice(ti * 128, (ti + 1) * 128)
            trL = psauxp.tile([128, E], F32, name="trL", tag="aux")
            nc.tensor.transpose(trL[:], lg_sb[:, sl], Id_sb[:E, :E])
            L = smallp.tile([128, E], F32, name="L", tag="L")
            nc.scalar.copy(L[:], trL[:])
            w_tiles.append(_chain(nc, smallp, L))

        # ---- phase B emission (PE order): base k-loops with the w-path
        # transposes/expansion slotted between the first tiles so PE never
        # waits on the DVE/ACT chains.
        def base_tile(ob, ti):
            tsl = slice(ti * 128, (ti + 1) * 128)
            pss = psmainp.tile([128, 512], F32, name=f"mm{ob}_{ti}", tag="mm")
            for k in range(KT):
                nc.tensor.matmul(
                    pss[:], xh_sb[:, k, tsl], W_sb[:, ob, k, :],
                    start=(k == 0), stop=False,
                )
            return pss

        def finish_tile(ob, ti, pss, veng):
            tsl = slice(ti * 128, (ti + 1) * 128)
            osl = slice(ob * 512, (ob + 1) * 512)
            nc.tensor.matmul(
                pss[:], us_sb[:, tsl], Bc_sb[:, osl], start=False, stop=True,
            )
            st = stagep.tile([128, 512], F32, name="st", tag="st")
            if veng:
                nc.vector.tensor_copy(st[:], pss[:])
            else:
                nc.scalar.copy(st[:], pss[:])
            nc.sync.dma_start(out[tsl, osl], st[:])

        def w_path():
            # transpose w back to [e, t], expand e -> er, scale u.
            for ti in range(TI):
                sl = slice(ti * 128, (ti + 1) * 128)
                trW = psauxp.tile([E, 128], F32, name="trW", tag="aux")
                nc.tensor.transpose(trW[:], w_tiles[ti][:], Id_sb[:])
                nc.scalar.copy(wT_sb[:, sl], trW[:])
            for tb in range(TBS):
                sl = slice(tb * 512, (tb + 1) * 512)
                wb_ps = psauxp.tile([ER, 512], F32, name="wbps", tag="aux")
                nc.tensor.matmul(wb_ps[:], Mm_sb[:], wT_sb[:, sl], start=True, stop=True)
                nc.scalar.copy(Wb_sb[:, sl], wb_ps[:])
                nc.vector.tensor_tensor(
                    us_sb[:, sl], u_ps[tb][:], Wb_sb[:, sl], ALU.mult,
                )

        # ob=0: experts lag 2 tiles behind their base loops so us (ready only
        # after the chains + w_path) never stalls PE.
        pend = {}
        pend[0] = base_tile(0, 0)
        pend[1] = base_tile(0, 1)
        w_path()
        for ti in range(2, TI):
            pend[ti] = base_tile(0, ti)
            finish_tile(0, ti - 2, pend.pop(ti - 2), veng=(ti % 2 == 0))
        finish_tile(0, TI - 2, pend.pop(TI - 2), veng=False)
        finish_tile(0, TI - 1, pend.pop(TI - 1), veng=True)
        for ob in range(1, OBS):
            for ti in range(TI):
                pss = base_tile(ob, ti)
                finish_tile(ob, ti, pss, veng=(ti % 2 == 0))


def build_nc():
    nc = bacc.Bacc("TRN2", target_bir_lowering=False, debug=False, num_devices=N_CORES)
    xh = nc.dram_tensor("xh", [D, NT], BF16, kind="ExternalInput").ap()
    WTo = [
        nc.dram_tensor(f"WTo{ob}", [128, KT, 512], BF16, kind="ExternalInput").ap()
        for ob in range(OBS)
    ]
    AT = nc.dram_tensor("AT", [128, KT, ER], BF16, kind="ExternalInput").ap()
    RT = nc.dram_tensor("RT", [128, KT, E], BF16, kind="ExternalInput").ap()
    Bc = nc.dram_tensor("Bc", [ER, O], BF16, kind="ExternalInput").ap()
    Mm = nc.dram_tensor("Mm", [E, ER], BF16, kind="ExternalInput").ap()
    Idn = nc.dram_tensor("Idn", [128, 128], F32, kind="ExternalInput").ap()
    out = nc.dram_tensor("out", [NT, O], F32, kind="ExternalOutput").ap()
    with tile.TileContext(nc) as tc:
        _body(tc, nc, xh, WTo, AT, RT, Bc, Mm, Idn, out)
    nc.compile()
    return nc


def get_nc():
    if "nc" not in _NC_CACHE:
        _NC_CACHE["nc"] = build_nc()
    return _NC_CACHE["nc"]


def make_in_maps(x, weight, lora_A, lora_B, router_w):
    x = np.ascontiguousarray(np.asarray(x, dtype=np.float32)).reshape(N_TOK, D)
    weight = np.asarray(weight, dtype=np.float32)
    lora_A = np.asarray(lora_A, dtype=np.float32)
    lora_B = np.asarray(lora_B, dtype=np.float32)
    router_w = np.asarray(router_w, dtype=np.float32)

    def to_pk(a):
        # [D, C] -> [128, KT, C]: partition p holds row k*128+p for each k chunk
        return np.ascontiguousarray(a.reshape(KT, 128, a.shape[1]).transpose(1, 0, 2))

    WT = weight.T.astype(ml_dtypes.bfloat16)  # [D, O]
    # o-block-major contiguous: WTo[ob] = [128, KT, 512] (partition, k-chunk, o)
    WTo = [
        np.ascontiguousarray(
            WT[:, ob * 512:(ob + 1) * 512].reshape(KT, 128, 512).transpose(1, 0, 2)
        )
        for ob in range(OBS)
    ]
    ATm = to_pk(lora_A.reshape(ER, D).T).astype(ml_dtypes.bfloat16)
    RTm = to_pk(np.ascontiguousarray(router_w.T)).astype(ml_dtypes.bfloat16)
    Bcm = np.ascontiguousarray(lora_B.transpose(0, 2, 1).reshape(ER, O)).astype(ml_dtypes.bfloat16)
    Mmm = np.zeros((E, ER), dtype=np.float32)
    for e in range(E):
        Mmm[e, e * R:(e + 1) * R] = 1.0
    Mmm = Mmm.astype(ml_dtypes.bfloat16)
    Idn = np.eye(128, dtype=np.float32)

    in_maps = []
    for c in range(N_CORES):
        xTc = np.ascontiguousarray(x[c * NT:(c + 1) * NT].T)
        xhc = xTc.astype(ml_dtypes.bfloat16)
        im = {
            "xh": xhc,
            "AT": ATm,
            "RT": RTm,
            "Bc": Bcm,
            "Mm": Mmm,
            "Idn": Idn,
        }
        for ob in range(OBS):
            im[f"WTo{ob}"] = WTo[ob]
        in_maps.append(im)
    return in_maps


def kernel(x, weight, lora_A, lora_B, router_w):
    global LAST_RESULTS
    from concourse.bass_utils import run_bass_kernel_spmd

    in_maps = make_in_maps(x, weight, lora_A, lora_B, router_w)
    nc = get_nc()
    trace = bool(os.environ.get("KBENCH_TRACE"))
    res = run_bass_kernel_spmd(nc, in_maps, core_ids=list(range(N_CORES)), trace=trace)
    LAST_RESULTS = res
    outs = [np.asarray(res.results[c]["out"], dtype=np.float32) for c in range(N_CORES)]
    return np.concatenate(outs, axis=0).reshape(4, 2048, 2048)


# revision 8
# speedup vs baseline: 1.0193x; 1.0193x over previous
"""LoRA-MoE layer (base dense + top-2 routed rank-16 LoRA experts) on 8 TRN2 cores.

Strategy: data-parallel over tokens (8192 tokens -> 1024/core), all weights
replicated, zero collectives. Per-core fused Bass/Tile kernel, v2:

  phase A (paced by the xh DMA stream, chunk by chunk):
    lg^T[e,t]  = R^T.xh     (single-pass bf16 — top-2 flips are ~16/8192
                             tokens and contribute ~4e-3 rel err, well
                             under the 2e-2 gate)
    u^T[er,t]  = A^T.xh     (bf16)
  phase A2: transpose lg to [t,8] tiles, top-2 softmax chains on DVE/ACT,
    transpose w back, expand to [er,t] via one-hot matmul, us = u*Wb*2.
  phase B (ob outer, ti inner): out[t,o] = xh^T.T @ W^T accumulated over
    16 k-chunks + us^T.T @ Bc into the same PSUM tile, then staged copy +
    DMA out.  W streams as 4 o-block-major 2 MiB DMAs timed to land just
    as phase B starts; out DMAs spread evenly so the tail is one tile.
"""

import os
import sys

import numpy as np


def _ensure_concourse():
    try:
        import concourse  # noqa: F401
    except ImportError:
        for p in ("/opt/trn_rl_repo", os.path.expanduser("~/.axon_site/_ro/trn_rl_repo")):
            if os.path.isdir(p):
                sys.path.insert(0, p)
                break


_ensure_concourse()

import ml_dtypes  # noqa: E402
import concourse.bass as bass  # noqa: E402,F401
import concourse.tile as tile  # noqa: E402
from concourse import bacc, mybir  # noqa: E402

F32 = mybir.dt.float32
BF16 = mybir.dt.bfloat16
X_AX = mybir.AxisListType.X
ALU = mybir.AluOpType
ACT = mybir.ActivationFunctionType

N_CORES = 8
N_TOK = 8192          # total tokens (4 x 2048)
NT = N_TOK // N_CORES  # tokens per core = 1024
D = 2048
O = 2048
E = 8
R = 16
ER = E * R            # 128
KT = D // 128         # 16 contraction chunks
TI = NT // 128        # 8 token tiles
OBS = 4               # o blocks of 512
TBS = 2               # token blocks of 512

_NC_CACHE = {}
LAST_RESULTS = None


def _chain(nc, smallp, L):
    """Top-2 softmax weights for one [128, E] logits tile; SCALING folded."""
    m1 = smallp.tile([128, 1], F32, name="m1", tag="m1")
    nc.vector.reduce_max(m1[:], L[:], axis=X_AX)
    nm1 = smallp.tile([128, 1], F32, name="nm1", tag="nm1")
    nc.scalar.mul(nm1[:], m1[:], -1.0)
    msk = smallp.tile([128, E], F32, name="msk", tag="msk")
    nc.vector.tensor_scalar(msk[:], L[:], m1[:], -1e30, ALU.is_equal, ALU.mult)
    L2 = smallp.tile([128, E], F32, name="L2", tag="L2")
    nc.vector.tensor_tensor(L2[:], L[:], msk[:], ALU.add)
    m2 = smallp.tile([128, 1], F32, name="m2", tag="m2")
    nc.vector.reduce_max(m2[:], L2[:], axis=X_AX)
    eL = smallp.tile([128, E], F32, name="eL", tag="eL")
    nc.scalar.activation(eL[:], L[:], ACT.Exp, bias=nm1[:])
    ge = smallp.tile([128, E], F32, name="ge", tag="ge")
    nc.vector.tensor_scalar(ge[:], L[:], m2[:], None, ALU.is_ge)
    un = smallp.tile([128, E], F32, name="un", tag="un")
    nc.vector.tensor_tensor(un[:], eL[:], ge[:], ALU.mult)
    s = smallp.tile([128, 1], F32, name="s", tag="s")
    nc.vector.reduce_sum(s[:], un[:], axis=X_AX)
    r = smallp.tile([128, 1], F32, name="r", tag="r")
    nc.vector.reciprocal(r[:], s[:])
    r2 = smallp.tile([128, 1], F32, name="r2", tag="r2")
    nc.scalar.mul(r2[:], r[:], 2.0)  # fold SCALING = 2.0
    w = smallp.tile([128, E], F32, name="w", tag="w", bufs=8)
    nc.vector.tensor_scalar(w[:], un[:], r2[:], None, ALU.mult)
    return w


def _body(tc, nc, xh, WTo, AT, RT, Bc, Mm, Idn, out):
    with (
        tc.tile_pool(name="const", bufs=1) as constp,
        tc.tile_pool(name="small", bufs=4) as smallp,
        tc.tile_pool(name="stage", bufs=4) as stagep,
        tc.tile_pool(name="ps_aux", bufs=2, space="PSUM") as psauxp,
        tc.tile_pool(name="ps_u", bufs=1, space="PSUM") as psup,
        tc.tile_pool(name="ps_main", bufs=4, space="PSUM") as psmainp,
    ):
        # ---- resident SBUF tensors ----
        xh_sb = constp.tile([128, KT, NT], BF16, name="xh_sb")
        W_sb = constp.tile([128, OBS, KT, 512], BF16, name="W_sb")
        AT_sb = constp.tile([128, KT, ER], BF16, name="AT_sb")
        RT_sb = constp.tile([128, KT, E], BF16, name="RT_sb")
        Bc_sb = constp.tile([ER, O], BF16, name="Bc_sb")
        Mm_sb = constp.tile([E, ER], BF16, name="Mm_sb")
        Id_sb = constp.tile([128, 128], F32, name="Id_sb")
        lg_sb = constp.tile([E, NT], F32, name="lg_sb")
        wT_sb = constp.tile([E, NT], BF16, name="wT_sb")
        Wb_sb = constp.tile([ER, NT], F32, name="Wb_sb")
        us_sb = constp.tile([ER, NT], BF16, name="us_sb")

        scr_sb = constp.tile([128, 512], BF16, name="scr_sb")

        # DMA order = consumption order on one HWDGE FIFO: router + first xh
        # chunks ahead of A (the LoRA-A matmuls lag 2 chunks), the rest of
        # the xh stream that paces phase A, then W in 4 o-block-major 2 MiB
        # transfers that land right as phase B begins; Bc/Mm slipped behind
        # W[0] (not needed until the first expert matmul / expansion).
        nc.sync.dma_start(RT_sb[:], RT[:])
        for k in range(2):
            nc.sync.dma_start(xh_sb[:, k, :], xh[k * 128:(k + 1) * 128, :])
        nc.sync.dma_start(AT_sb[:], AT[:])
        for k in range(2, KT):
            nc.sync.dma_start(xh_sb[:, k, :], xh[k * 128:(k + 1) * 128, :])
        nc.sync.dma_start(Id_sb[:], Idn[:])
        nc.sync.dma_start(W_sb[:, 0, :, :], WTo[0])
        nc.sync.dma_start(Bc_sb[:], Bc[:])
        nc.sync.dma_start(Mm_sb[:], Mm[:])
        for ob in range(1, OBS):
            nc.sync.dma_start(W_sb[:, ob, :, :], WTo[ob])

        # HAM warmup: ~3.4us of junk matmuls during the DMA lead-in so the
        # PE clock gate opens before real work arrives.
        nc.vector.memset(scr_sb[:], 0)
        wu_ps = psmainp.tile([128, 128], F32, name="wu", tag="mm")
        for _ in range(28):
            nc.tensor.matmul(
                wu_ps[:], scr_sb[:, :128], scr_sb[:, :128],
                start=True, stop=True,
            )

        # ---- phase A: router logits + LoRA-A projection, chunk-paced ----
        lg_ps = [psauxp.tile([E, 512], F32, name=f"lgps{tb}", tag="aux") for tb in range(TBS)]
        u_ps = [psup.tile([ER, 512], F32, name=f"ups{tb}", tag=f"u{tb}") for tb in range(TBS)]

        def u_mms(k):
            for tb in range(TBS):
                nc.tensor.matmul(
                    u_ps[tb][:], AT_sb[:, k, :], xh_sb[:, k, tb * 512:(tb + 1) * 512],
                    start=(k == 0), stop=(k == KT - 1),
                )

        for k in range(KT):
            for tb in range(TBS):
                nc.tensor.matmul(
                    lg_ps[tb][:], RT_sb[:, k, :], xh_sb[:, k, tb * 512:(tb + 1) * 512],
                    start=(k == 0), stop=(k == KT - 1),
                )
            if k >= 2:
                u_mms(k - 2)
        u_mms(KT - 2)
        u_mms(KT - 1)
        for tb in range(TBS):
            nc.scalar.copy(lg_sb[:, tb * 512:(tb + 1) * 512], lg_ps[tb][:])

        # ---- phase A2a: transpose logits to [t, 8] tiles + softmax chains ----
        w_tiles = []
        for ti in range(TI):
            sl = slice(ti * 128, (ti + 1) * 128)
            trL = psauxp.tile([128, E], F32, name="trL", tag="aux")
            nc.tensor.transpose(trL[:], lg_sb[:, sl], Id_sb[:E, :E])
            L = smallp.tile([128, E], F32, name="L", tag="L")
            nc.scalar.copy(L[:], trL[:])
            w_tiles.append(_chain(nc, smallp, L))

        # ---- phase B emission (PE order): base k-loops with the w-path
        # transposes/expansion slotted between the first tiles so PE never
        # waits on the DVE/ACT chains.
        def base_tile(ob, ti):
            tsl = slice(ti * 128, (ti + 1) * 128)
            pss = psmainp.tile([128, 512], F32, name=f"mm{ob}_{ti}", tag="mm")
            for k in range(KT):
                nc.tensor.matmul(
                    pss[:], xh_sb[:, k, tsl], W_sb[:, ob, k, :],
                    start=(k == 0), stop=False,
                )
            return pss

        def finish_tile(ob, ti, pss, veng):
            tsl = slice(ti * 128, (ti + 1) * 128)
            osl = slice(ob * 512, (ob + 1) * 512)
            nc.tensor.matmul(
                pss[:], us_sb[:, tsl], Bc_sb[:, osl], start=False, stop=True,
            )
            st = stagep.tile([128, 512], BF16, name="st", tag="st")
            if veng:
                nc.vector.tensor_copy(st[:], pss[:])
            else:
                nc.scalar.copy(st[:], pss[:])
            nc.sync.dma_start(out[tsl, osl], st[:])

        def w_path():
            # transpose w back to [e, t], expand e -> er, scale u.
            for ti in range(TI):
                sl = slice(ti * 128, (ti + 1) * 128)
                trW = psauxp.tile([E, 128], F32, name="trW", tag="aux")
                nc.tensor.transpose(trW[:], w_tiles[ti][:], Id_sb[:])
                nc.scalar.copy(wT_sb[:, sl], trW[:])
            for tb in range(TBS):
                sl = slice(tb * 512, (tb + 1) * 512)
                wb_ps = psauxp.tile([ER, 512], F32, name="wbps", tag="aux")
                nc.tensor.matmul(wb_ps[:], Mm_sb[:], wT_sb[:, sl], start=True, stop=True)
                nc.scalar.copy(Wb_sb[:, sl], wb_ps[:])
                nc.vector.tensor_tensor(
                    us_sb[:, sl], u_ps[tb][:], Wb_sb[:, sl], ALU.mult,
                )

        # ob=0: experts lag 2 tiles behind their base loops so us (ready only
        # after the chains + w_path) never stalls PE.
        pend = {}
        pend[0] = base_tile(0, 0)
        pend[1] = base_tile(0, 1)
        w_path()
        for ti in range(2, TI):
            pend[ti] = base_tile(0, ti)
            finish_tile(0, ti - 2, pend.pop(ti - 2), veng=(ti % 2 == 0))
        finish_tile(0, TI - 2, pend.pop(TI - 2), veng=False)
        finish_tile(0, TI - 1, pend.pop(TI - 1), veng=True)
        for ob in range(1, OBS):
            for ti in range(TI):
                pss = base_tile(ob, ti)
                finish_tile(ob, ti, pss, veng=(ti % 2 == 0))


def build_nc():
    nc = bacc.Bacc("TRN2", target_bir_lowering=False, debug=False, num_devices=N_CORES)
    xh = nc.dram_tensor("xh", [D, NT], BF16, kind="ExternalInput").ap()
    WTo = [
        nc.dram_tensor(f"WTo{ob}", [128, KT, 512], BF16, kind="ExternalInput").ap()
        for ob in range(OBS)
    ]
    AT = nc.dram_tensor("AT", [128, KT, ER], BF16, kind="ExternalInput").ap()
    RT = nc.dram_tensor("RT", [128, KT, E], BF16, kind="ExternalInput").ap()
    Bc = nc.dram_tensor("Bc", [ER, O], BF16, kind="ExternalInput").ap()
    Mm = nc.dram_tensor("Mm", [E, ER], BF16, kind="ExternalInput").ap()
    Idn = nc.dram_tensor("Idn", [128, 128], F32, kind="ExternalInput").ap()
    out = nc.dram_tensor("out", [NT, O], BF16, kind="ExternalOutput").ap()
    with tile.TileContext(nc) as tc:
        _body(tc, nc, xh, WTo, AT, RT, Bc, Mm, Idn, out)
    nc.compile()
    return nc


def get_nc():
    if "nc" not in _NC_CACHE:
        _NC_CACHE["nc"] = build_nc()
    return _NC_CACHE["nc"]


def make_in_maps(x, weight, lora_A, lora_B, router_w):
    x = np.ascontiguousarray(np.asarray(x, dtype=np.float32)).reshape(N_TOK, D)
    weight = np.asarray(weight, dtype=np.float32)
    lora_A = np.asarray(lora_A, dtype=np.float32)
    lora_B = np.asarray(lora_B, dtype=np.float32)
    router_w = np.asarray(router_w, dtype=np.float32)

    def to_pk(a):
        # [D, C] -> [128, KT, C]: partition p holds row k*128+p for each k chunk
        return np.ascontiguousarray(a.reshape(KT, 128, a.shape[1]).transpose(1, 0, 2))

    WT = weight.T.astype(ml_dtypes.bfloat16)  # [D, O]
    # o-block-major contiguous: WTo[ob] = [128, KT, 512] (partition, k-chunk, o)
    WTo = [
        np.ascontiguousarray(
            WT[:, ob * 512:(ob + 1) * 512].reshape(KT, 128, 512).transpose(1, 0, 2)
        )
        for ob in range(OBS)
    ]
    ATm = to_pk(lora_A.reshape(ER, D).T).astype(ml_dtypes.bfloat16)
    RTm = to_pk(np.ascontiguousarray(router_w.T)).astype(ml_dtypes.bfloat16)
    Bcm = np.ascontiguousarray(lora_B.transpose(0, 2, 1).reshape(ER, O)).astype(ml_dtypes.bfloat16)
    Mmm = np.zeros((E, ER), dtype=np.float32)
    for e in range(E):
        Mmm[e, e * R:(e + 1) * R] = 1.0
    Mmm = Mmm.astype(ml_dtypes.bfloat16)
    Idn = np.eye(128, dtype=np.float32)

    in_maps = []
    for c in range(N_CORES):
        xTc = np.ascontiguousarray(x[c * NT:(c + 1) * NT].T)
        xhc = xTc.astype(ml_dtypes.bfloat16)
        im = {
            "xh": xhc,
            "AT": ATm,
            "RT": RTm,
            "Bc": Bcm,
            "Mm": Mmm,
            "Idn": Idn,
        }
        for ob in range(OBS):
            im[f"WTo{ob}"] = WTo[ob]
        in_maps.append(im)
    return in_maps


def kernel(x, weight, lora_A, lora_B, router_w):
    global LAST_RESULTS
    from concourse.bass_utils import run_bass_kernel_spmd

    in_maps = make_in_maps(x, weight, lora_A, lora_B, router_w)
    nc = get_nc()
    trace = bool(os.environ.get("KBENCH_TRACE"))
    res = run_bass_kernel_spmd(nc, in_maps, core_ids=list(range(N_CORES)), trace=trace)
    LAST_RESULTS = res
    outs = [np.asarray(res.results[c]["out"], dtype=np.float32) for c in range(N_CORES)]
    return np.concatenate(outs, axis=0).reshape(4, 2048, 2048)


# revision 18
# speedup vs baseline: 1.0766x; 1.0562x over previous
"""LoRA-MoE layer (base dense + top-2 routed rank-16 LoRA experts) on 8 TRN2 cores.

Strategy: data-parallel over tokens (8192 tokens -> 1024/core), all weights
replicated, zero collectives. Per-core fused Bass/Tile kernel, v2:

  phase A (paced by the xh DMA stream, chunk by chunk):
    lg^T[e,t]  = R^T.xh     (single-pass bf16 — top-2 flips are ~16/8192
                             tokens and contribute ~4e-3 rel err, well
                             under the 2e-2 gate)
    u^T[er,t]  = A^T.xh     (bf16)
  phase A2: transpose lg to [t,8] tiles, top-2 softmax chains on DVE/ACT,
    transpose w back, expand to [er,t] via one-hot matmul, us = u*Wb*2.
  phase B (ob outer, ti inner): out[t,o] = xh^T.T @ W^T accumulated over
    16 k-chunks + us^T.T @ Bc into the same PSUM tile, then staged copy +
    DMA out.  W streams as 4 o-block-major 2 MiB DMAs timed to land just
    as phase B starts; out DMAs spread evenly so the tail is one tile.
"""

import os
import sys

import numpy as np


def _ensure_concourse():
    try:
        import concourse  # noqa: F401
    except ImportError:
        for p in ("/opt/trn_rl_repo", os.path.expanduser("~/.axon_site/_ro/trn_rl_repo")):
            if os.path.isdir(p):
                sys.path.insert(0, p)
                break


_ensure_concourse()

import ml_dtypes  # noqa: E402
import concourse.bass as bass  # noqa: E402,F401
import concourse.tile as tile  # noqa: E402
from concourse import bacc, mybir  # noqa: E402

F32 = mybir.dt.float32
BF16 = mybir.dt.bfloat16
X_AX = mybir.AxisListType.X
ALU = mybir.AluOpType
ACT = mybir.ActivationFunctionType

N_CORES = 8
N_TOK = 8192          # total tokens (4 x 2048)
NT = N_TOK // N_CORES  # tokens per core = 1024
D = 2048
O = 2048
E = 8
R = 16
ER = E * R            # 128
KT = D // 128         # 16 contraction chunks
TI = NT // 128        # 8 token tiles
OBS = 4               # o blocks of 512
TBS = 2               # token blocks of 512

_NC_CACHE = {}
LAST_RESULTS = None


G = 16  # 32-col groups per 512-token block (stream-transpose granularity)


def _batched_chain(nc, smallp, L_all, tb):
    """Top-2 softmax weights for one 512-token block, all tokens at once.

    L_all is [32, G*32] f32: partition a = token-within-32-group, free col
    32j+b = (group j, expert b); only b<8 is real (rest is the M=32 router
    padding).  Writes the weights (SCALING folded) back in-place over the
    b<8 columns.
    """
    L = L_all[:].rearrange("p (j b) -> p j b", b=32)[:, :, :E]
    m1 = smallp.tile([32, G], F32, name="m1", tag=f"m1_{tb}")
    nc.vector.reduce_max(m1[:], L, axis=X_AX)
    m1b = m1[:].unsqueeze(2).broadcast_to([32, G, E])
    d = smallp.tile([32, G, E], F32, name="d", tag=f"d_{tb}")
    nc.vector.tensor_tensor(d[:], L, m1b, ALU.subtract)
    # mask the top-1 entry to -1e30, then find the 2nd max
    msk = smallp.tile([32, G, E], F32, name="msk", tag=f"msk_{tb}")
    nc.vector.tensor_scalar(msk[:], d[:], 0.0, -1e30, ALU.is_equal, ALU.mult)
    L2 = smallp.tile([32, G, E], F32, name="L2", tag=f"L2_{tb}")
    nc.vector.tensor_tensor(L2[:], d[:], msk[:], ALU.add)
    m2 = smallp.tile([32, G], F32, name="m2", tag=f"m2_{tb}")
    nc.vector.reduce_max(m2[:], L2[:], axis=X_AX)
    eL = smallp.tile([32, G, E], F32, name="eL", tag=f"eL_{tb}")
    nc.scalar.activation(eL[:], d[:], ACT.Exp)
    ge = smallp.tile([32, G, E], F32, name="ge", tag=f"ge_{tb}")
    nc.vector.tensor_tensor(ge[:], d[:], m2[:].unsqueeze(2).broadcast_to([32, G, E]), ALU.is_ge)
    un = smallp.tile([32, G, E], F32, name="un", tag=f"un_{tb}")
    nc.vector.tensor_tensor(un[:], eL[:], ge[:], ALU.mult)
    s = smallp.tile([32, G], F32, name="s", tag=f"s_{tb}")
    nc.vector.reduce_sum(s[:], un[:], axis=X_AX)
    r = smallp.tile([32, G], F32, name="r", tag=f"r_{tb}")
    nc.vector.reciprocal(r[:], s[:])
    r2 = smallp.tile([32, G], F32, name="r2", tag=f"r2_{tb}")
    nc.scalar.mul(r2[:], r[:], 2.0)  # fold SCALING = 2.0
    nc.vector.tensor_tensor(
        L, un[:], r2[:].unsqueeze(2).broadcast_to([32, G, E]), ALU.mult,
    )


def _body(tc, nc, xh, WTo, AT, RT, Bc, Mm, out):
    with (
        tc.tile_pool(name="const", bufs=1) as constp,
        tc.tile_pool(name="small", bufs=4) as smallp,
        tc.tile_pool(name="stage", bufs=4) as stagep,
        tc.tile_pool(name="ps_aux", bufs=2, space="PSUM") as psauxp,
        tc.tile_pool(name="ps_u", bufs=1, space="PSUM") as psup,
        tc.tile_pool(name="ps_main", bufs=4, space="PSUM") as psmainp,
    ):
        # ---- resident SBUF tensors ----
        xh_sb = constp.tile([128, KT, NT], BF16, name="xh_sb")
        W_sb = constp.tile([128, OBS, KT, 512], BF16, name="W_sb")
        AT_sb = constp.tile([128, KT, ER], BF16, name="AT_sb")
        RT_sb = constp.tile([128, KT, 32], BF16, name="RT_sb")
        Bc_sb = constp.tile([ER, O], BF16, name="Bc_sb")
        Mm_sb = constp.tile([E, ER], BF16, name="Mm_sb")
        L_all = [constp.tile([32, 512], F32, name=f"L_all{tb}") for tb in range(TBS)]
        wTp = [constp.tile([32, 512], F32, name=f"wTp{tb}") for tb in range(TBS)]
        wTb = [constp.tile([E, 512], BF16, name=f"wTb{tb}") for tb in range(TBS)]
        Wb_sb = constp.tile([ER, NT], F32, name="Wb_sb")
        us_sb = constp.tile([ER, NT], BF16, name="us_sb")

        scr_sb = constp.tile([128, 512], BF16, name="scr_sb")

        # DMA order = consumption order on one HWDGE FIFO: router + first xh
        # chunks ahead of A (the LoRA-A matmuls lag 2 chunks), the rest of
        # the xh stream that paces phase A, then W in 4 o-block-major 2 MiB
        # transfers that land right as phase B begins; Bc/Mm slipped behind
        # W[0] (not needed until the first expert matmul / expansion).
        nc.sync.dma_start(RT_sb[:], RT[:])
        for k in range(2):
            nc.sync.dma_start(xh_sb[:, k, :], xh[k * 128:(k + 1) * 128, :])
        nc.sync.dma_start(AT_sb[:], AT[:])
        for k in range(2, KT):
            nc.sync.dma_start(xh_sb[:, k, :], xh[k * 128:(k + 1) * 128, :])
        nc.sync.dma_start(Mm_sb[:], Mm[:])
        nc.sync.dma_start(W_sb[:, 0, :, :], WTo[0])
        nc.sync.dma_start(Bc_sb[:], Bc[:])
        for ob in range(1, OBS):
            nc.sync.dma_start(W_sb[:, ob, :, :], WTo[ob])

        # HAM warmup: ~3.4us of junk matmuls during the DMA lead-in so the
        # PE clock gate opens before real work arrives.
        nc.vector.memset(scr_sb[:], 0)
        wu_ps = psmainp.tile([128, 128], F32, name="wu", tag="mm")
        for _ in range(28):
            nc.tensor.matmul(
                wu_ps[:], scr_sb[:, :128], scr_sb[:, :128],
                start=True, stop=True,
            )

        # ---- phase A: router logits + LoRA-A projection, chunk-paced.
        # Emitted in chunk pairs (2x lg then 2x u, lagged) so the new-chunk
        # semaphore wait and PSUM bank switches amortize over 4-MM groups.
        lg_ps = [psauxp.tile([32, 512], F32, name=f"lgps{tb}", tag="aux") for tb in range(TBS)]
        u_ps = [psup.tile([ER, 512], F32, name=f"ups{tb}", tag=f"u{tb}") for tb in range(TBS)]

        def lg_mms(k):
            for tb in range(TBS):
                nc.tensor.matmul(
                    lg_ps[tb][:], RT_sb[:, k, :], xh_sb[:, k, tb * 512:(tb + 1) * 512],
                    start=(k == 0), stop=(k == KT - 1),
                )

        def u_mms(k):
            for tb in range(TBS):
                nc.tensor.matmul(
                    u_ps[tb][:], AT_sb[:, k, :], xh_sb[:, k, tb * 512:(tb + 1) * 512],
                    start=(k == 0), stop=(k == KT - 1),
                )

        for kk in range(0, KT, 2):
            lg_mms(kk)
            lg_mms(kk + 1)
            if kk >= 2:
                u_mms(kk - 2)
                u_mms(kk - 1)
        u_mms(KT - 2)
        u_mms(KT - 1)

        # ---- phase A2: stream-transpose logits to token-major, batched
        # top-2 softmax chain, stream-transpose the weights back (all on
        # DVE/ACT; PE only does the one-hot expansion matmuls).
        for tb in range(TBS):
            nc.vector.transpose(L_all[tb][:], lg_ps[tb][:])
            _batched_chain(nc, smallp, L_all[tb], tb)
            nc.vector.transpose(wTp[tb][:], L_all[tb][:])
            nc.scalar.copy(wTb[tb][:], wTp[tb][:E, :])

        # ---- phase B emission (PE order): base k-loops with the w-path
        # transposes/expansion slotted between the first tiles so PE never
        # waits on the DVE/ACT chains.
        def base_tile(ob, ti):
            tsl = slice(ti * 128, (ti + 1) * 128)
            pss = psmainp.tile([128, 512], F32, name=f"mm{ob}_{ti}", tag="mm")
            for k in range(KT):
                nc.tensor.matmul(
                    pss[:], xh_sb[:, k, tsl], W_sb[:, ob, k, :],
                    start=(k == 0), stop=False,
                )
            return pss

        def finish_tile(ob, ti, pss, veng):
            tsl = slice(ti * 128, (ti + 1) * 128)
            osl = slice(ob * 512, (ob + 1) * 512)
            nc.tensor.matmul(
                pss[:], us_sb[:, tsl], Bc_sb[:, osl], start=False, stop=True,
            )
            st = stagep.tile([128, 512], BF16, name="st", tag="st")
            if veng:
                nc.vector.tensor_copy(st[:], pss[:])
            else:
                nc.scalar.copy(st[:], pss[:])
            nc.sync.dma_start(out[tsl, osl], st[:])

        def w_path():
            # expand the per-expert weights e -> er via one-hot matmul, scale u.
            for tb in range(TBS):
                sl = slice(tb * 512, (tb + 1) * 512)
                wb_ps = psauxp.tile([ER, 512], F32, name="wbps", tag="aux")
                nc.tensor.matmul(wb_ps[:], Mm_sb[:], wTb[tb][:], start=True, stop=True)
                nc.scalar.copy(Wb_sb[:, sl], wb_ps[:])
                nc.vector.tensor_tensor(
                    us_sb[:, sl], u_ps[tb][:], Wb_sb[:, sl], ALU.mult,
                )

        # ob=0: experts lag 2 tiles behind their base loops so us (ready only
        # after the chains + w_path) never stalls PE.
        pend = {}
        pend[0] = base_tile(0, 0)
        pend[1] = base_tile(0, 1)
        w_path()
        for ti in range(2, TI):
            pend[ti] = base_tile(0, ti)
            finish_tile(0, ti - 2, pend.pop(ti - 2), veng=(ti % 2 == 0))
        finish_tile(0, TI - 2, pend.pop(TI - 2), veng=False)
        finish_tile(0, TI - 1, pend.pop(TI - 1), veng=True)
        for ob in range(1, OBS):
            for ti in range(TI):
                pss = base_tile(ob, ti)
                finish_tile(ob, ti, pss, veng=(ti % 2 == 0))


def build_nc():
    nc = bacc.Bacc("TRN2", target_bir_lowering=False, debug=False, num_devices=N_CORES)
    xh = nc.dram_tensor("xh", [D, NT], BF16, kind="ExternalInput").ap()
    WTo = [
        nc.dram_tensor(f"WTo{ob}", [128, KT, 512], BF16, kind="ExternalInput").ap()
        for ob in range(OBS)
    ]
    AT = nc.dram_tensor("AT", [128, KT, ER], BF16, kind="ExternalInput").ap()
    RT = nc.dram_tensor("RT", [128, KT, 32], BF16, kind="ExternalInput").ap()
    Bc = nc.dram_tensor("Bc", [ER, O], BF16, kind="ExternalInput").ap()
    Mm = nc.dram_tensor("Mm", [E, ER], BF16, kind="ExternalInput").ap()
    out = nc.dram_tensor("out", [NT, O], BF16, kind="ExternalOutput").ap()
    with tile.TileContext(nc) as tc:
        _body(tc, nc, xh, WTo, AT, RT, Bc, Mm, out)
    nc.compile()
    return nc


def get_nc():
    if "nc" not in _NC_CACHE:
        _NC_CACHE["nc"] = build_nc()
    return _NC_CACHE["nc"]


def make_in_maps(x, weight, lora_A, lora_B, router_w):
    x = np.ascontiguousarray(np.asarray(x, dtype=np.float32)).reshape(N_TOK, D)
    weight = np.asarray(weight, dtype=np.float32)
    lora_A = np.asarray(lora_A, dtype=np.float32)
    lora_B = np.asarray(lora_B, dtype=np.float32)
    router_w = np.asarray(router_w, dtype=np.float32)

    def to_pk(a):
        # [D, C] -> [128, KT, C]: partition p holds row k*128+p for each k chunk
        return np.ascontiguousarray(a.reshape(KT, 128, a.shape[1]).transpose(1, 0, 2))

    WT = weight.T.astype(ml_dtypes.bfloat16)  # [D, O]
    # o-block-major contiguous: WTo[ob] = [128, KT, 512] (partition, k-chunk, o)
    WTo = [
        np.ascontiguousarray(
            WT[:, ob * 512:(ob + 1) * 512].reshape(KT, 128, 512).transpose(1, 0, 2)
        )
        for ob in range(OBS)
    ]
    ATm = to_pk(lora_A.reshape(ER, D).T).astype(ml_dtypes.bfloat16)
    # router padded to 32 output cols (zeros) so logits land in a [32, 512]
    # PSUM tile that the 32x32 DVE stream-transpose can consume directly
    Rpad = np.zeros((D, 32), dtype=np.float32)
    Rpad[:, :E] = router_w.T
    RTm = to_pk(Rpad).astype(ml_dtypes.bfloat16)
    Bcm = np.ascontiguousarray(lora_B.transpose(0, 2, 1).reshape(ER, O)).astype(ml_dtypes.bfloat16)
    Mmm = np.zeros((E, ER), dtype=np.float32)
    for e in range(E):
        Mmm[e, e * R:(e + 1) * R] = 1.0
    Mmm = Mmm.astype(ml_dtypes.bfloat16)

    in_maps = []
    for c in range(N_CORES):
        xTc = np.ascontiguousarray(x[c * NT:(c + 1) * NT].T)
        xhc = xTc.astype(ml_dtypes.bfloat16)
        im = {
            "xh": xhc,
            "AT": ATm,
            "RT": RTm,
            "Bc": Bcm,
            "Mm": Mmm,
        }
        for ob in range(OBS):
            im[f"WTo{ob}"] = WTo[ob]
        in_maps.append(im)
    return in_maps


def kernel(x, weight, lora_A, lora_B, router_w):
    global LAST_RESULTS
    from concourse.bass_utils import run_bass_kernel_spmd

    in_maps = make_in_maps(x, weight, lora_A, lora_B, router_w)
    nc = get_nc()
    trace = bool(os.environ.get("KBENCH_TRACE"))
    res = run_bass_kernel_spmd(nc, in_maps, core_ids=list(range(N_CORES)), trace=trace)
    LAST_RESULTS = res
    outs = [np.asarray(res.results[c]["out"], dtype=np.float32) for c in range(N_CORES)]
    return np.concatenate(outs, axis=0).reshape(4, 2048, 2048)


# revision 20
# speedup vs baseline: 1.0842x; 1.0071x over previous
"""LoRA-MoE layer (base dense + top-2 routed rank-16 LoRA experts) on 8 TRN2 cores.

Strategy: data-parallel over tokens (8192 tokens -> 1024/core), all weights
replicated, zero collectives. Per-core fused Bass/Tile kernel, v2:

  phase A (paced by the xh DMA stream, chunk by chunk):
    lg^T[e,t]  = R^T.xh     (single-pass bf16 — top-2 flips are ~16/8192
                             tokens and contribute ~4e-3 rel err, well
                             under the 2e-2 gate)
    u^T[er,t]  = A^T.xh     (bf16)
  phase A2: transpose lg to [t,8] tiles, top-2 softmax chains on DVE/ACT,
    transpose w back, expand to [er,t] via one-hot matmul, us = u*Wb*2.
  phase B (ob outer, ti inner): out[t,o] = xh^T.T @ W^T accumulated over
    16 k-chunks + us^T.T @ Bc into the same PSUM tile, then staged copy +
    DMA out.  W streams as 4 o-block-major 2 MiB DMAs timed to land just
    as phase B starts; out DMAs spread evenly so the tail is one tile.
"""

import os
import sys

import numpy as np


def _ensure_concourse():
    try:
        import concourse  # noqa: F401
    except ImportError:
        for p in ("/opt/trn_rl_repo", os.path.expanduser("~/.axon_site/_ro/trn_rl_repo")):
            if os.path.isdir(p):
                sys.path.insert(0, p)
                break


_ensure_concourse()

import ml_dtypes  # noqa: E402
import concourse.bass as bass  # noqa: E402,F401
import concourse.tile as tile  # noqa: E402
from concourse import bacc, mybir  # noqa: E402

F32 = mybir.dt.float32
BF16 = mybir.dt.bfloat16
X_AX = mybir.AxisListType.X
ALU = mybir.AluOpType
ACT = mybir.ActivationFunctionType

N_CORES = 8
N_TOK = 8192          # total tokens (4 x 2048)
NT = N_TOK // N_CORES  # tokens per core = 1024
D = 2048
O = 2048
E = 8
R = 16
ER = E * R            # 128
KT = D // 128         # 16 contraction chunks
TI = NT // 128        # 8 token tiles
OBS = 4               # o blocks of 512
TBS = 2               # token blocks of 512

_NC_CACHE = {}
LAST_RESULTS = None


G = 16  # 32-col groups per 512-token block (stream-transpose granularity)


def _batched_chain(nc, smallp, L_all, tb):
    """Top-2 softmax weights for one 512-token block, all tokens at once.

    L_all is [32, G*32] f32: partition a = token-within-32-group, free col
    32j+b = (group j, expert b); only b<8 is real (rest is the M=32 router
    padding).  Writes the weights (SCALING folded) back in-place over the
    b<8 columns.
    """
    L = L_all[:].rearrange("p (j b) -> p j b", b=32)[:, :, :E]
    m1 = smallp.tile([32, G], F32, name="m1", tag=f"m1_{tb}")
    nc.vector.reduce_max(m1[:], L, axis=X_AX)
    m1b = m1[:].unsqueeze(2).broadcast_to([32, G, E])
    d = smallp.tile([32, G, E], F32, name="d", tag=f"d_{tb}")
    nc.vector.tensor_tensor(d[:], L, m1b, ALU.subtract)
    # mask the top-1 entry to -1e30, then find the 2nd max
    msk = smallp.tile([32, G, E], F32, name="msk", tag=f"msk_{tb}")
    nc.vector.tensor_scalar(msk[:], d[:], 0.0, -1e30, ALU.is_equal, ALU.mult)
    L2 = smallp.tile([32, G, E], F32, name="L2", tag=f"L2_{tb}")
    nc.vector.tensor_tensor(L2[:], d[:], msk[:], ALU.add)
    m2 = smallp.tile([32, G], F32, name="m2", tag=f"m2_{tb}")
    nc.vector.reduce_max(m2[:], L2[:], axis=X_AX)
    eL = smallp.tile([32, G, E], F32, name="eL", tag=f"eL_{tb}")
    nc.scalar.activation(eL[:], d[:], ACT.Exp)
    ge = smallp.tile([32, G, E], F32, name="ge", tag=f"ge_{tb}")
    nc.vector.tensor_tensor(ge[:], d[:], m2[:].unsqueeze(2).broadcast_to([32, G, E]), ALU.is_ge)
    un = smallp.tile([32, G, E], F32, name="un", tag=f"un_{tb}")
    nc.vector.tensor_tensor(un[:], eL[:], ge[:], ALU.mult)
    s = smallp.tile([32, G], F32, name="s", tag=f"s_{tb}")
    nc.vector.reduce_sum(s[:], un[:], axis=X_AX)
    r = smallp.tile([32, G], F32, name="r", tag=f"r_{tb}")
    nc.vector.reciprocal(r[:], s[:])
    r2 = smallp.tile([32, G], F32, name="r2", tag=f"r2_{tb}")
    nc.scalar.mul(r2[:], r[:], 2.0)  # fold SCALING = 2.0
    nc.vector.tensor_tensor(
        L, un[:], r2[:].unsqueeze(2).broadcast_to([32, G, E]), ALU.mult,
    )


def _body(tc, nc, xh, WTo, AT, RT, Bc, Mm, out):
    with (
        tc.tile_pool(name="const", bufs=1) as constp,
        tc.tile_pool(name="small", bufs=4) as smallp,
        tc.tile_pool(name="stage", bufs=4) as stagep,
        tc.tile_pool(name="ps_aux", bufs=2, space="PSUM") as psauxp,
        tc.tile_pool(name="ps_u", bufs=1, space="PSUM") as psup,
        tc.tile_pool(name="ps_main", bufs=4, space="PSUM") as psmainp,
    ):
        # ---- resident SBUF tensors ----
        xh_sb = constp.tile([128, KT, NT], BF16, name="xh_sb")
        W_sb = constp.tile([128, OBS, KT, 512], BF16, name="W_sb")
        AT_sb = constp.tile([128, KT, ER], BF16, name="AT_sb")
        RT_sb = constp.tile([128, KT, 32], BF16, name="RT_sb")
        Bc_sb = constp.tile([ER, O], BF16, name="Bc_sb")
        Mm_sb = constp.tile([E, ER], BF16, name="Mm_sb")
        L_all = [constp.tile([32, 512], F32, name=f"L_all{tb}") for tb in range(TBS)]
        wTp = [constp.tile([32, 512], F32, name=f"wTp{tb}") for tb in range(TBS)]
        wTb = [constp.tile([E, 512], BF16, name=f"wTb{tb}") for tb in range(TBS)]
        Wb_sb = constp.tile([ER, NT], F32, name="Wb_sb")
        us_sb = constp.tile([ER, NT], BF16, name="us_sb")

        scr_sb = constp.tile([128, 512], BF16, name="scr_sb")

        # DMA order = consumption order on one HWDGE FIFO: router + first xh
        # chunks ahead of A (the LoRA-A matmuls lag 2 chunks), the rest of
        # the xh stream that paces phase A, then W in 4 o-block-major 2 MiB
        # transfers that land right as phase B begins; Bc/Mm slipped behind
        # W[0] (not needed until the first expert matmul / expansion).
        nc.sync.dma_start(RT_sb[:], RT[:])
        for k in range(KT):
            nc.sync.dma_start(xh_sb[:, k, :], xh[k * 128:(k + 1) * 128, :])
            if k in (1, 3, 5, 7):
                a = (k - 1) * 2
                nc.sync.dma_start(AT_sb[:, a:a + 4, :], AT[:, a:a + 4, :])
        nc.sync.dma_start(Mm_sb[:], Mm[:])
        nc.sync.dma_start(W_sb[:, 0, :, :], WTo[0])
        nc.sync.dma_start(Bc_sb[:], Bc[:])
        for ob in range(1, OBS):
            nc.sync.dma_start(W_sb[:, ob, :, :], WTo[ob])

        # HAM warmup: ~3.4us of junk matmuls during the DMA lead-in so the
        # PE clock gate opens before real work arrives.
        nc.vector.memset(scr_sb[:], 0)
        wu_ps = psmainp.tile([128, 128], F32, name="wu", tag="mm")
        for _ in range(20):
            nc.tensor.matmul(
                wu_ps[:], scr_sb[:, :128], scr_sb[:, :128],
                start=True, stop=True,
            )

        # ---- phase A: router logits + LoRA-A projection, chunk-paced.
        # Emitted in chunk pairs (2x lg then 2x u, lagged) so the new-chunk
        # semaphore wait and PSUM bank switches amortize over 4-MM groups.
        lg_ps = [psauxp.tile([32, 512], F32, name=f"lgps{tb}", tag="aux") for tb in range(TBS)]
        u_ps = [psup.tile([ER, 512], F32, name=f"ups{tb}", tag=f"u{tb}") for tb in range(TBS)]

        def lg_mms(k):
            for tb in range(TBS):
                nc.tensor.matmul(
                    lg_ps[tb][:], RT_sb[:, k, :], xh_sb[:, k, tb * 512:(tb + 1) * 512],
                    start=(k == 0), stop=(k == KT - 1),
                )

        def u_mms(k):
            for tb in range(TBS):
                nc.tensor.matmul(
                    u_ps[tb][:], AT_sb[:, k, :], xh_sb[:, k, tb * 512:(tb + 1) * 512],
                    start=(k == 0), stop=(k == KT - 1),
                )

        for kk in range(0, KT, 2):
            lg_mms(kk)
            lg_mms(kk + 1)
            if kk >= 2:
                u_mms(kk - 2)
                u_mms(kk - 1)
        u_mms(KT - 2)
        u_mms(KT - 1)

        # ---- phase A2: stream-transpose logits to token-major, batched
        # top-2 softmax chain, stream-transpose the weights back (all on
        # DVE/ACT; PE only does the one-hot expansion matmuls).
        for tb in range(TBS):
            nc.vector.transpose(L_all[tb][:], lg_ps[tb][:])
            _batched_chain(nc, smallp, L_all[tb], tb)
            nc.vector.transpose(wTp[tb][:], L_all[tb][:])
            nc.scalar.copy(wTb[tb][:], wTp[tb][:E, :])

        # ---- phase B emission (PE order): base k-loops with the w-path
        # transposes/expansion slotted between the first tiles so PE never
        # waits on the DVE/ACT chains.
        def base_tile(ob, ti):
            tsl = slice(ti * 128, (ti + 1) * 128)
            pss = psmainp.tile([128, 512], F32, name=f"mm{ob}_{ti}", tag="mm")
            for k in range(KT):
                nc.tensor.matmul(
                    pss[:], xh_sb[:, k, tsl], W_sb[:, ob, k, :],
                    start=(k == 0), stop=False,
                )
            return pss

        def finish_tile(ob, ti, pss, veng):
            tsl = slice(ti * 128, (ti + 1) * 128)
            osl = slice(ob * 512, (ob + 1) * 512)
            nc.tensor.matmul(
                pss[:], us_sb[:, tsl], Bc_sb[:, osl], start=False, stop=True,
            )
            st = stagep.tile([128, 512], BF16, name="st", tag="st")
            if veng:
                nc.vector.tensor_copy(st[:], pss[:])
            else:
                nc.scalar.copy(st[:], pss[:])
            nc.sync.dma_start(out[tsl, osl], st[:])

        def w_path():
            # expand the per-expert weights e -> er via one-hot matmul, scale u.
            for tb in range(TBS):
                sl = slice(tb * 512, (tb + 1) * 512)
                wb_ps = psauxp.tile([ER, 512], F32, name="wbps", tag="aux")
                nc.tensor.matmul(wb_ps[:], Mm_sb[:], wTb[tb][:], start=True, stop=True)
                nc.scalar.copy(Wb_sb[:, sl], wb_ps[:])
                nc.vector.tensor_tensor(
                    us_sb[:, sl], u_ps[tb][:], Wb_sb[:, sl], ALU.mult,
                )

        # ob=0: experts lag 2 tiles behind their base loops so us (ready only
        # after the chains + w_path) never stalls PE.
        pend = {}
        pend[0] = base_tile(0, 0)
        pend[1] = base_tile(0, 1)
        w_path()
        for ti in range(2, TI):
            pend[ti] = base_tile(0, ti)
            finish_tile(0, ti - 2, pend.pop(ti - 2), veng=(ti % 2 == 0))
        finish_tile(0, TI - 2, pend.pop(TI - 2), veng=False)
        finish_tile(0, TI - 1, pend.pop(TI - 1), veng=True)
        for ob in range(1, OBS):
            for ti in range(TI):
                pss = base_tile(ob, ti)
                finish_tile(ob, ti, pss, veng=(ti % 2 == 0))


def build_nc():
    nc = bacc.Bacc("TRN2", target_bir_lowering=False, debug=False, num_devices=N_CORES)
    xh = nc.dram_tensor("xh", [D, NT], BF16, kind="ExternalInput").ap()
    WTo = [
        nc.dram_tensor(f"WTo{ob}", [128, KT, 512], BF16, kind="ExternalInput").ap()
        for ob in range(OBS)
    ]
    AT = nc.dram_tensor("AT", [128, KT, ER], BF16, kind="ExternalInput").ap()
    RT = nc.dram_tensor("RT", [128, KT, 32], BF16, kind="ExternalInput").ap()
    Bc = nc.dram_tensor("Bc", [ER, O], BF16, kind="ExternalInput").ap()
    Mm = nc.dram_tensor("Mm", [E, ER], BF16, kind="ExternalInput").ap()
    out = nc.dram_tensor("out", [NT, O], BF16, kind="ExternalOutput").ap()
    with tile.TileContext(nc) as tc:
        _body(tc, nc, xh, WTo, AT, RT, Bc, Mm, out)
    nc.compile()
    return nc


def get_nc():
    if "nc" not in _NC_CACHE:
        _NC_CACHE["nc"] = build_nc()
    return _NC_CACHE["nc"]


def make_in_maps(x, weight, lora_A, lora_B, router_w):
    x = np.ascontiguousarray(np.asarray(x, dtype=np.float32)).reshape(N_TOK, D)
    weight = np.asarray(weight, dtype=np.float32)
    lora_A = np.asarray(lora_A, dtype=np.float32)
    lora_B = np.asarray(lora_B, dtype=np.float32)
    router_w = np.asarray(router_w, dtype=np.float32)

    def to_pk(a):
        # [D, C] -> [128, KT, C]: partition p holds row k*128+p for each k chunk
        return np.ascontiguousarray(a.reshape(KT, 128, a.shape[1]).transpose(1, 0, 2))

    WT = weight.T.astype(ml_dtypes.bfloat16)  # [D, O]
    # o-block-major contiguous: WTo[ob] = [128, KT, 512] (partition, k-chunk, o)
    WTo = [
        np.ascontiguousarray(
            WT[:, ob * 512:(ob + 1) * 512].reshape(KT, 128, 512).transpose(1, 0, 2)
        )
        for ob in range(OBS)
    ]
    ATm = to_pk(lora_A.reshape(ER, D).T).astype(ml_dtypes.bfloat16)
    # router padded to 32 output cols (zeros) so logits land in a [32, 512]
    # PSUM tile that the 32x32 DVE stream-transpose can consume directly
    Rpad = np.zeros((D, 32), dtype=np.float32)
    Rpad[:, :E] = router_w.T
    RTm = to_pk(Rpad).astype(ml_dtypes.bfloat16)
    Bcm = np.ascontiguousarray(lora_B.transpose(0, 2, 1).reshape(ER, O)).astype(ml_dtypes.bfloat16)
    Mmm = np.zeros((E, ER), dtype=np.float32)
    for e in range(E):
        Mmm[e, e * R:(e + 1) * R] = 1.0
    Mmm = Mmm.astype(ml_dtypes.bfloat16)

    in_maps = []
    for c in range(N_CORES):
        xTc = np.ascontiguousarray(x[c * NT:(c + 1) * NT].T)
        xhc = xTc.astype(ml_dtypes.bfloat16)
        im = {
            "xh": xhc,
            "AT": ATm,
            "RT": RTm,
            "Bc": Bcm,
            "Mm": Mmm,
        }
        for ob in range(OBS):
            im[f"WTo{ob}"] = WTo[ob]
        in_maps.append(im)
    return in_maps


def kernel(x, weight, lora_A, lora_B, router_w):
    global LAST_RESULTS
    from concourse.bass_utils import run_bass_kernel_spmd

    in_maps = make_in_maps(x, weight, lora_A, lora_B, router_w)
    nc = get_nc()
    trace = bool(os.environ.get("KBENCH_TRACE"))
    res = run_bass_kernel_spmd(nc, in_maps, core_ids=list(range(N_CORES)), trace=trace)
    LAST_RESULTS = res
    outs = [np.asarray(res.results[c]["out"], dtype=np.float32) for c in range(N_CORES)]
    return np.concatenate(outs, axis=0).reshape(4, 2048, 2048)


# revision 22
# speedup vs baseline: 1.0922x; 1.0074x over previous
"""LoRA-MoE layer (base dense + top-2 routed rank-16 LoRA experts) on 8 TRN2 cores.

Strategy: data-parallel over tokens (8192 tokens -> 1024/core), all weights
replicated, zero collectives. Per-core fused Bass/Tile kernel, v2:

  phase A (paced by the xh DMA stream, chunk by chunk):
    lg^T[e,t]  = R^T.xh     (single-pass bf16 — top-2 flips are ~16/8192
                             tokens and contribute ~4e-3 rel err, well
                             under the 2e-2 gate)
    u^T[er,t]  = A^T.xh     (bf16)
  phase A2: transpose lg to [t,8] tiles, top-2 softmax chains on DVE/ACT,
    transpose w back, expand to [er,t] via one-hot matmul, us = u*Wb*2.
  phase B (ob outer, ti inner): out[t,o] = xh^T.T @ W^T accumulated over
    16 k-chunks + us^T.T @ Bc into the same PSUM tile, then staged copy +
    DMA out.  W streams as 4 o-block-major 2 MiB DMAs timed to land just
    as phase B starts; out DMAs spread evenly so the tail is one tile.
"""

import os
import sys

import numpy as np


def _ensure_concourse():
    try:
        import concourse  # noqa: F401
    except ImportError:
        for p in ("/opt/trn_rl_repo", os.path.expanduser("~/.axon_site/_ro/trn_rl_repo")):
            if os.path.isdir(p):
                sys.path.insert(0, p)
                break


_ensure_concourse()

import ml_dtypes  # noqa: E402
import concourse.bass as bass  # noqa: E402,F401
import concourse.tile as tile  # noqa: E402
from concourse import bacc, mybir  # noqa: E402

F32 = mybir.dt.float32
BF16 = mybir.dt.bfloat16
X_AX = mybir.AxisListType.X
ALU = mybir.AluOpType
ACT = mybir.ActivationFunctionType

N_CORES = 8
N_TOK = 8192          # total tokens (4 x 2048)
NT = N_TOK // N_CORES  # tokens per core = 1024
D = 2048
O = 2048
E = 8
R = 16
ER = E * R            # 128
KT = D // 128         # 16 contraction chunks
TI = NT // 128        # 8 token tiles
OBS = 4               # o blocks of 512
TBS = 2               # token blocks of 512

_NC_CACHE = {}
LAST_RESULTS = None


G = 16  # 32-col groups per 512-token block (stream-transpose granularity)


def _batched_chain(nc, smallp, L_all, tb):
    """Top-2 softmax weights for one 512-token block, all tokens at once.

    L_all is [32, G*32] f32: partition a = token-within-32-group, free col
    32j+b = (group j, expert b); only b<8 is real (rest is the M=32 router
    padding).  Writes the weights (SCALING folded) back in-place over the
    b<8 columns.
    """
    L = L_all[:].rearrange("p (j b) -> p j b", b=32)[:, :, :E]
    m1 = smallp.tile([32, G], F32, name="m1", tag=f"m1_{tb}")
    nc.vector.reduce_max(m1[:], L, axis=X_AX)
    m1b = m1[:].unsqueeze(2).broadcast_to([32, G, E])
    d = smallp.tile([32, G, E], F32, name="d", tag=f"d_{tb}")
    nc.vector.tensor_tensor(d[:], L, m1b, ALU.subtract)
    # mask the top-1 entry to -1e30, then find the 2nd max
    msk = smallp.tile([32, G, E], F32, name="msk", tag=f"msk_{tb}")
    nc.vector.tensor_scalar(msk[:], d[:], 0.0, -1e30, ALU.is_equal, ALU.mult)
    L2 = smallp.tile([32, G, E], F32, name="L2", tag=f"L2_{tb}")
    nc.vector.tensor_tensor(L2[:], d[:], msk[:], ALU.add)
    m2 = smallp.tile([32, G], F32, name="m2", tag=f"m2_{tb}")
    nc.vector.reduce_max(m2[:], L2[:], axis=X_AX)
    eL = smallp.tile([32, G, E], F32, name="eL", tag=f"eL_{tb}")
    nc.scalar.activation(eL[:], d[:], ACT.Exp)
    ge = smallp.tile([32, G, E], F32, name="ge", tag=f"ge_{tb}")
    nc.vector.tensor_tensor(ge[:], d[:], m2[:].unsqueeze(2).broadcast_to([32, G, E]), ALU.is_ge)
    un = smallp.tile([32, G, E], F32, name="un", tag=f"un_{tb}")
    nc.vector.tensor_tensor(un[:], eL[:], ge[:], ALU.mult)
    s = smallp.tile([32, G], F32, name="s", tag=f"s_{tb}")
    nc.vector.reduce_sum(s[:], un[:], axis=X_AX)
    r = smallp.tile([32, G], F32, name="r", tag=f"r_{tb}")
    nc.vector.reciprocal(r[:], s[:])
    r2 = smallp.tile([32, G], F32, name="r2", tag=f"r2_{tb}")
    nc.scalar.mul(r2[:], r[:], 2.0)  # fold SCALING = 2.0
    nc.vector.tensor_tensor(
        L, un[:], r2[:].unsqueeze(2).broadcast_to([32, G, E]), ALU.mult,
    )


def _body(tc, nc, xh, WTo, AT, RT, Bc, Mm, out):
    with (
        tc.tile_pool(name="const", bufs=1) as constp,
        tc.tile_pool(name="small", bufs=4) as smallp,
        tc.tile_pool(name="stage", bufs=4) as stagep,
        tc.tile_pool(name="ps_aux", bufs=2, space="PSUM") as psauxp,
        tc.tile_pool(name="ps_u", bufs=1, space="PSUM") as psup,
        tc.tile_pool(name="ps_main", bufs=4, space="PSUM") as psmainp,
    ):
        # ---- resident SBUF tensors ----
        xh_sb = constp.tile([128, KT, NT], BF16, name="xh_sb")
        W_sb = constp.tile([128, OBS, KT, 512], BF16, name="W_sb")
        AT_sb = constp.tile([128, KT, ER], BF16, name="AT_sb")
        RT_sb = constp.tile([128, KT, 32], BF16, name="RT_sb")
        Bc_sb = constp.tile([ER, O], BF16, name="Bc_sb")
        Mm_sb = constp.tile([E, ER], BF16, name="Mm_sb")
        L_all = [constp.tile([32, 512], F32, name=f"L_all{tb}") for tb in range(TBS)]
        wTp = [constp.tile([32, 512], F32, name=f"wTp{tb}") for tb in range(TBS)]
        wTb = [constp.tile([E, 512], BF16, name=f"wTb{tb}") for tb in range(TBS)]
        Wb_sb = constp.tile([ER, NT], F32, name="Wb_sb")
        us_sb = constp.tile([ER, NT], BF16, name="us_sb")

        scr_sb = constp.tile([128, 512], BF16, name="scr_sb")

        # DMA order = consumption order on one HWDGE FIFO: router + first xh
        # chunks ahead of A (the LoRA-A matmuls lag 2 chunks), the rest of
        # the xh stream that paces phase A, then W in 4 o-block-major 2 MiB
        # transfers that land right as phase B begins; Bc/Mm slipped behind
        # W[0] (not needed until the first expert matmul / expansion).
        nc.sync.dma_start(RT_sb[:], RT[:])
        for k in range(KT):
            nc.sync.dma_start(xh_sb[:, k, :], xh[k * 128:(k + 1) * 128, :])
            if k in (1, 3, 5, 7):
                a = (k - 1) * 2
                nc.sync.dma_start(AT_sb[:, a:a + 4, :], AT[:, a:a + 4, :])
        nc.sync.dma_start(Mm_sb[:], Mm[:])
        # W[0] split along k so base(ob=0, ti=0) starts on the first piece
        # instead of waiting out the whole 2 MiB transfer.
        for kp in range(0, KT, 4):
            nc.sync.dma_start(W_sb[:, 0, kp:kp + 4, :], WTo[0][:, kp:kp + 4, :])
        nc.sync.dma_start(Bc_sb[:], Bc[:])
        for ob in range(1, OBS):
            nc.sync.dma_start(W_sb[:, ob, :, :], WTo[ob])

        # HAM warmup: ~3.4us of junk matmuls during the DMA lead-in so the
        # PE clock gate opens before real work arrives.
        nc.vector.memset(scr_sb[:], 0)
        wu_ps = psmainp.tile([128, 128], F32, name="wu", tag="mm")
        for _ in range(26):
            nc.tensor.matmul(
                wu_ps[:], scr_sb[:, :128], scr_sb[:, :128],
                start=True, stop=True,
            )

        # ---- phase A: router logits + LoRA-A projection, chunk-paced.
        # Emitted in chunk pairs (2x lg then 2x u, lagged) so the new-chunk
        # semaphore wait and PSUM bank switches amortize over 4-MM groups.
        lg_ps = [psauxp.tile([32, 512], F32, name=f"lgps{tb}", tag="aux") for tb in range(TBS)]
        u_ps = [psup.tile([ER, 512], F32, name=f"ups{tb}", tag=f"u{tb}") for tb in range(TBS)]

        def lg_mms(k):
            for tb in range(TBS):
                nc.tensor.matmul(
                    lg_ps[tb][:], RT_sb[:, k, :], xh_sb[:, k, tb * 512:(tb + 1) * 512],
                    start=(k == 0), stop=(k == KT - 1),
                )

        def u_mms(k):
            for tb in range(TBS):
                nc.tensor.matmul(
                    u_ps[tb][:], AT_sb[:, k, :], xh_sb[:, k, tb * 512:(tb + 1) * 512],
                    start=(k == 0), stop=(k == KT - 1),
                )

        for kk in range(0, KT, 2):
            lg_mms(kk)
            lg_mms(kk + 1)
            if kk >= 2:
                u_mms(kk - 2)
                u_mms(kk - 1)
        u_mms(KT - 2)
        u_mms(KT - 1)

        # ---- phase A2: stream-transpose logits to token-major, batched
        # top-2 softmax chain, stream-transpose the weights back (all on
        # DVE/ACT; PE only does the one-hot expansion matmuls).
        for tb in range(TBS):
            nc.vector.transpose(L_all[tb][:], lg_ps[tb][:])
            _batched_chain(nc, smallp, L_all[tb], tb)
            nc.vector.transpose(wTp[tb][:], L_all[tb][:])
            nc.scalar.copy(wTb[tb][:], wTp[tb][:E, :])

        # ---- phase B emission (PE order): base k-loops with the w-path
        # transposes/expansion slotted between the first tiles so PE never
        # waits on the DVE/ACT chains.
        def base_tile(ob, ti):
            tsl = slice(ti * 128, (ti + 1) * 128)
            pss = psmainp.tile([128, 512], F32, name=f"mm{ob}_{ti}", tag="mm")
            for k in range(KT):
                nc.tensor.matmul(
                    pss[:], xh_sb[:, k, tsl], W_sb[:, ob, k, :],
                    start=(k == 0), stop=False,
                )
            return pss

        def finish_tile(ob, ti, pss, veng):
            tsl = slice(ti * 128, (ti + 1) * 128)
            osl = slice(ob * 512, (ob + 1) * 512)
            nc.tensor.matmul(
                pss[:], us_sb[:, tsl], Bc_sb[:, osl], start=False, stop=True,
            )
            st = stagep.tile([128, 512], BF16, name="st", tag="st")
            if veng:
                nc.vector.tensor_copy(st[:], pss[:])
            else:
                nc.scalar.copy(st[:], pss[:])
            nc.sync.dma_start(out[tsl, osl], st[:])

        def w_path():
            # expand the per-expert weights e -> er via one-hot matmul, scale u.
            for tb in range(TBS):
                sl = slice(tb * 512, (tb + 1) * 512)
                wb_ps = psauxp.tile([ER, 512], F32, name="wbps", tag="aux")
                nc.tensor.matmul(wb_ps[:], Mm_sb[:], wTb[tb][:], start=True, stop=True)
                nc.scalar.copy(Wb_sb[:, sl], wb_ps[:])
                nc.vector.tensor_tensor(
                    us_sb[:, sl], u_ps[tb][:], Wb_sb[:, sl], ALU.mult,
                )

        # ob=0: experts lag 2 tiles behind their base loops so us (ready only
        # after the chains + w_path) never stalls PE.
        pend = {}
        pend[0] = base_tile(0, 0)
        pend[1] = base_tile(0, 1)
        w_path()
        for ti in range(2, TI):
            pend[ti] = base_tile(0, ti)
            finish_tile(0, ti - 2, pend.pop(ti - 2), veng=(ti % 2 == 0))
        finish_tile(0, TI - 2, pend.pop(TI - 2), veng=False)
        finish_tile(0, TI - 1, pend.pop(TI - 1), veng=True)
        for ob in range(1, OBS):
            for ti in range(TI):
                pss = base_tile(ob, ti)
                finish_tile(ob, ti, pss, veng=(ti % 2 == 0))


def build_nc():
    nc = bacc.Bacc("TRN2", target_bir_lowering=False, debug=False, num_devices=N_CORES)
    xh = nc.dram_tensor("xh", [D, NT], BF16, kind="ExternalInput").ap()
    WTo = [
        nc.dram_tensor(f"WTo{ob}", [128, KT, 512], BF16, kind="ExternalInput").ap()
        for ob in range(OBS)
    ]
    AT = nc.dram_tensor("AT", [128, KT, ER], BF16, kind="ExternalInput").ap()
    RT = nc.dram_tensor("RT", [128, KT, 32], BF16, kind="ExternalInput").ap()
    Bc = nc.dram_tensor("Bc", [ER, O], BF16, kind="ExternalInput").ap()
    Mm = nc.dram_tensor("Mm", [E, ER], BF16, kind="ExternalInput").ap()
    out = nc.dram_tensor("out", [NT, O], BF16, kind="ExternalOutput").ap()
    with tile.TileContext(nc) as tc:
        _body(tc, nc, xh, WTo, AT, RT, Bc, Mm, out)
    nc.compile()
    return nc


def get_nc():
    if "nc" not in _NC_CACHE:
        _NC_CACHE["nc"] = build_nc()
    return _NC_CACHE["nc"]


def make_in_maps(x, weight, lora_A, lora_B, router_w):
    x = np.ascontiguousarray(np.asarray(x, dtype=np.float32)).reshape(N_TOK, D)
    weight = np.asarray(weight, dtype=np.float32)
    lora_A = np.asarray(lora_A, dtype=np.float32)
    lora_B = np.asarray(lora_B, dtype=np.float32)
    router_w = np.asarray(router_w, dtype=np.float32)

    def to_pk(a):
        # [D, C] -> [128, KT, C]: partition p holds row k*128+p for each k chunk
        return np.ascontiguousarray(a.reshape(KT, 128, a.shape[1]).transpose(1, 0, 2))

    WT = weight.T.astype(ml_dtypes.bfloat16)  # [D, O]
    # o-block-major contiguous: WTo[ob] = [128, KT, 512] (partition, k-chunk, o)
    WTo = [
        np.ascontiguousarray(
            WT[:, ob * 512:(ob + 1) * 512].reshape(KT, 128, 512).transpose(1, 0, 2)
        )
        for ob in range(OBS)
    ]
    ATm = to_pk(lora_A.reshape(ER, D).T).astype(ml_dtypes.bfloat16)
    # router padded to 32 output cols (zeros) so logits land in a [32, 512]
    # PSUM tile that the 32x32 DVE stream-transpose can consume directly
    Rpad = np.zeros((D, 32), dtype=np.float32)
    Rpad[:, :E] = router_w.T
    RTm = to_pk(Rpad).astype(ml_dtypes.bfloat16)
    Bcm = np.ascontiguousarray(lora_B.transpose(0, 2, 1).reshape(ER, O)).astype(ml_dtypes.bfloat16)
    Mmm = np.zeros((E, ER), dtype=np.float32)
    for e in range(E):
        Mmm[e, e * R:(e + 1) * R] = 1.0
    Mmm = Mmm.astype(ml_dtypes.bfloat16)

    in_maps = []
    for c in range(N_CORES):
        xTc = np.ascontiguousarray(x[c * NT:(c + 1) * NT].T)
        xhc = xTc.astype(ml_dtypes.bfloat16)
        im = {
            "xh": xhc,
            "AT": ATm,
            "RT": RTm,
            "Bc": Bcm,
            "Mm": Mmm,
        }
        for ob in range(OBS):
            im[f"WTo{ob}"] = WTo[ob]
        in_maps.append(im)
    return in_maps


def kernel(x, weight, lora_A, lora_B, router_w):
    global LAST_RESULTS
    from concourse.bass_utils import run_bass_kernel_spmd

    in_maps = make_in_maps(x, weight, lora_A, lora_B, router_w)
    nc = get_nc()
    trace = bool(os.environ.get("KBENCH_TRACE"))
    res = run_bass_kernel_spmd(nc, in_maps, core_ids=list(range(N_CORES)), trace=trace)
    LAST_RESULTS = res
    outs = [np.asarray(res.results[c]["out"], dtype=np.float32) for c in range(N_CORES)]
    return np.concatenate(outs, axis=0).reshape(4, 2048, 2048)


# revision 25
# speedup vs baseline: 1.0961x; 1.0035x over previous
"""LoRA-MoE layer (base dense + top-2 routed rank-16 LoRA experts) on 8 TRN2 cores.

Strategy: data-parallel over tokens (8192 tokens -> 1024/core), all weights
replicated, zero collectives. Per-core fused Bass/Tile kernel, v2:

  phase A (paced by the xh DMA stream, chunk by chunk):
    lg^T[e,t]  = R^T.xh     (single-pass bf16 — top-2 flips are ~16/8192
                             tokens and contribute ~4e-3 rel err, well
                             under the 2e-2 gate)
    u^T[er,t]  = A^T.xh     (bf16)
  phase A2: transpose lg to [t,8] tiles, top-2 softmax chains on DVE/ACT,
    transpose w back, expand to [er,t] via one-hot matmul, us = u*Wb*2.
  phase B (ob outer, ti inner): out[t,o] = xh^T.T @ W^T accumulated over
    16 k-chunks + us^T.T @ Bc into the same PSUM tile, then staged copy +
    DMA out.  W streams as 4 o-block-major 2 MiB DMAs timed to land just
    as phase B starts; out DMAs spread evenly so the tail is one tile.
"""

import os
import sys

import numpy as np


def _ensure_concourse():
    try:
        import concourse  # noqa: F401
    except ImportError:
        for p in ("/opt/trn_rl_repo", os.path.expanduser("~/.axon_site/_ro/trn_rl_repo")):
            if os.path.isdir(p):
                sys.path.insert(0, p)
                break


_ensure_concourse()

import ml_dtypes  # noqa: E402
import concourse.bass as bass  # noqa: E402,F401
import concourse.tile as tile  # noqa: E402
from concourse import bacc, mybir  # noqa: E402

F32 = mybir.dt.float32
BF16 = mybir.dt.bfloat16
X_AX = mybir.AxisListType.X
ALU = mybir.AluOpType
ACT = mybir.ActivationFunctionType

N_CORES = 8
N_TOK = 8192          # total tokens (4 x 2048)
NT = N_TOK // N_CORES  # tokens per core = 1024
D = 2048
O = 2048
E = 8
R = 16
ER = E * R            # 128
KT = D // 128         # 16 contraction chunks
TI = NT // 128        # 8 token tiles
OBS = 4               # o blocks of 512
TBS = 2               # token blocks of 512

_NC_CACHE = {}
LAST_RESULTS = None


G = 16  # 32-col groups per 512-token block (stream-transpose granularity)


def _batched_chain(nc, smallp, L_all, tb):
    """Top-2 softmax weights for one 512-token block, all tokens at once.

    L_all is [32, G*32] f32: partition a = token-within-32-group, free col
    32j+b = (group j, expert b); only b<8 is real (rest is the M=32 router
    padding).  Writes the weights (SCALING folded) back in-place over the
    b<8 columns.
    """
    L = L_all[:].rearrange("p (j b) -> p j b", b=32)[:, :, :E]
    m1 = smallp.tile([32, G], F32, name="m1", tag=f"m1_{tb}")
    nc.vector.reduce_max(m1[:], L, axis=X_AX)
    m1b = m1[:].unsqueeze(2).broadcast_to([32, G, E])
    d = smallp.tile([32, G, E], F32, name="d", tag=f"d_{tb}")
    nc.vector.tensor_tensor(d[:], L, m1b, ALU.subtract)
    # mask the top-1 entry to -1e30, then find the 2nd max
    msk = smallp.tile([32, G, E], F32, name="msk", tag=f"msk_{tb}")
    nc.vector.tensor_scalar(msk[:], d[:], 0.0, -1e30, ALU.is_equal, ALU.mult)
    L2 = smallp.tile([32, G, E], F32, name="L2", tag=f"L2_{tb}")
    nc.vector.tensor_tensor(L2[:], d[:], msk[:], ALU.add)
    m2 = smallp.tile([32, G], F32, name="m2", tag=f"m2_{tb}")
    nc.vector.reduce_max(m2[:], L2[:], axis=X_AX)
    eL = smallp.tile([32, G, E], F32, name="eL", tag=f"eL_{tb}")
    nc.scalar.activation(eL[:], d[:], ACT.Exp)
    ge = smallp.tile([32, G, E], F32, name="ge", tag=f"ge_{tb}")
    nc.vector.tensor_tensor(ge[:], d[:], m2[:].unsqueeze(2).broadcast_to([32, G, E]), ALU.is_ge)
    un = smallp.tile([32, G, E], F32, name="un", tag=f"un_{tb}")
    nc.vector.tensor_tensor(un[:], eL[:], ge[:], ALU.mult)
    s = smallp.tile([32, G], F32, name="s", tag=f"s_{tb}")
    nc.vector.reduce_sum(s[:], un[:], axis=X_AX)
    r = smallp.tile([32, G], F32, name="r", tag=f"r_{tb}")
    nc.vector.reciprocal(r[:], s[:])
    r2 = smallp.tile([32, G], F32, name="r2", tag=f"r2_{tb}")
    nc.scalar.mul(r2[:], r[:], 2.0)  # fold SCALING = 2.0
    nc.vector.tensor_tensor(
        L, un[:], r2[:].unsqueeze(2).broadcast_to([32, G, E]), ALU.mult,
    )


def _body(tc, nc, xh, WTo, AT, RT, Bc, Mm, out):
    with (
        tc.tile_pool(name="const", bufs=1) as constp,
        tc.tile_pool(name="small", bufs=4) as smallp,
        tc.tile_pool(name="stage", bufs=4) as stagep,
        tc.tile_pool(name="ps_aux", bufs=2, space="PSUM") as psauxp,
        tc.tile_pool(name="ps_u", bufs=1, space="PSUM") as psup,
        tc.tile_pool(name="ps_main", bufs=4, space="PSUM") as psmainp,
    ):
        # ---- resident SBUF tensors ----
        xh_sb = constp.tile([128, KT, NT], BF16, name="xh_sb")
        W_sb = constp.tile([128, OBS, KT, 512], BF16, name="W_sb")
        AT_sb = constp.tile([128, KT, ER], BF16, name="AT_sb")
        RT_sb = constp.tile([128, KT, 32], BF16, name="RT_sb")
        Bc_sb = constp.tile([ER, O], BF16, name="Bc_sb")
        Mm_sb = constp.tile([E, ER], BF16, name="Mm_sb")
        L_all = [constp.tile([32, 512], F32, name=f"L_all{tb}") for tb in range(TBS)]
        wTp = [constp.tile([32, 512], F32, name=f"wTp{tb}") for tb in range(TBS)]
        wTb = [constp.tile([E, 512], BF16, name=f"wTb{tb}") for tb in range(TBS)]
        Wb_sb = constp.tile([ER, NT], F32, name="Wb_sb")
        us_sb = constp.tile([ER, NT], BF16, name="us_sb")

        scr_sb = constp.tile([128, 512], BF16, name="scr_sb")

        # DMA order = consumption order on one HWDGE FIFO: router + first xh
        # chunks ahead of A (the LoRA-A matmuls lag 2 chunks), the rest of
        # the xh stream that paces phase A, then W in 4 o-block-major 2 MiB
        # transfers that land right as phase B begins; Bc/Mm slipped behind
        # W[0] (not needed until the first expert matmul / expansion).
        nc.sync.dma_start(RT_sb[:], RT[:])
        for k in range(KT):
            nc.sync.dma_start(xh_sb[:, k, :], xh[k * 128:(k + 1) * 128, :])
            if k in (1, 3, 5, 7):
                a = (k - 1) * 2
                nc.sync.dma_start(AT_sb[:, a:a + 4, :], AT[:, a:a + 4, :])
        nc.sync.dma_start(Mm_sb[:], Mm[:])
        # W[0] split along k so base(ob=0, ti=0) starts on the first piece
        # instead of waiting out the whole 2 MiB transfer.
        for kp in range(0, KT, 4):
            nc.sync.dma_start(W_sb[:, 0, kp:kp + 4, :], WTo[0][:, kp:kp + 4, :])
        nc.sync.dma_start(Bc_sb[:], Bc[:])
        for ob in range(1, OBS):
            nc.sync.dma_start(W_sb[:, ob, :, :], WTo[ob])

        # HAM warmup: ~3.4us of junk matmuls during the DMA lead-in so the
        # PE clock gate opens before real work arrives.
        nc.vector.memset(scr_sb[:], 0)
        wu_ps = psmainp.tile([128, 128], F32, name="wu", tag="mm")
        for _ in range(26):
            nc.tensor.matmul(
                wu_ps[:], scr_sb[:, :128], scr_sb[:, :128],
                start=True, stop=True,
            )

        # ---- phase A: router logits + LoRA-A projection, chunk-paced.
        # Emitted in chunk pairs (2x lg then 2x u, lagged) so the new-chunk
        # semaphore wait and PSUM bank switches amortize over 4-MM groups.
        lg_ps = [psauxp.tile([32, 512], F32, name=f"lgps{tb}", tag="aux") for tb in range(TBS)]
        u_ps = [psup.tile([ER, 512], F32, name=f"ups{tb}", tag=f"u{tb}") for tb in range(TBS)]

        def lg_mms(k):
            for tb in range(TBS):
                nc.tensor.matmul(
                    lg_ps[tb][:], RT_sb[:, k, :], xh_sb[:, k, tb * 512:(tb + 1) * 512],
                    start=(k == 0), stop=(k == KT - 1),
                )

        def u_mms(k):
            for tb in range(TBS):
                nc.tensor.matmul(
                    u_ps[tb][:], AT_sb[:, k, :], xh_sb[:, k, tb * 512:(tb + 1) * 512],
                    start=(k == 0), stop=(k == KT - 1),
                )

        for kk in range(0, KT, 2):
            if kk >= 2:
                u_mms(kk - 2)
                u_mms(kk - 1)
            lg_mms(kk)
            lg_mms(kk + 1)
        u_mms(KT - 2)
        u_mms(KT - 1)

        # ---- phase A2: stream-transpose logits to token-major, batched
        # top-2 softmax chain, stream-transpose the weights back (all on
        # DVE/ACT; PE only does the one-hot expansion matmuls).
        for tb in range(TBS):
            nc.vector.transpose(L_all[tb][:], lg_ps[tb][:])
            _batched_chain(nc, smallp, L_all[tb], tb)
            nc.vector.transpose(wTp[tb][:], L_all[tb][:])
            nc.scalar.copy(wTb[tb][:], wTp[tb][:E, :])

        # ---- phase B emission (PE order): base k-loops with the w-path
        # transposes/expansion slotted between the first tiles so PE never
        # waits on the DVE/ACT chains.
        def base_tile(ob, ti):
            tsl = slice(ti * 128, (ti + 1) * 128)
            pss = psmainp.tile([128, 512], F32, name=f"mm{ob}_{ti}", tag="mm")
            for k in range(KT):
                nc.tensor.matmul(
                    pss[:], xh_sb[:, k, tsl], W_sb[:, ob, k, :],
                    start=(k == 0), stop=False,
                )
            return pss

        def finish_tile(ob, ti, pss, veng, last=False):
            tsl = slice(ti * 128, (ti + 1) * 128)
            osl = slice(ob * 512, (ob + 1) * 512)
            nc.tensor.matmul(
                pss[:], us_sb[:, tsl], Bc_sb[:, osl], start=False, stop=True,
            )
            st = stagep.tile([128, 512], BF16, name="st", tag="st")
            if last:
                # split the final drain across both copy engines so the
                # kernel tail is one half-tile copy + DMA, not a full one
                nc.scalar.copy(st[:, :256], pss[:, :256])
                nc.vector.tensor_copy(st[:, 256:], pss[:, 256:])
                nc.sync.dma_start(out[tsl, ob * 512:ob * 512 + 256], st[:, :256])
                nc.sync.dma_start(out[tsl, ob * 512 + 256:(ob + 1) * 512], st[:, 256:])
                return
            if veng:
                nc.vector.tensor_copy(st[:], pss[:])
            else:
                nc.scalar.copy(st[:], pss[:])
            nc.sync.dma_start(out[tsl, osl], st[:])

        def w_path():
            # expand the per-expert weights e -> er via one-hot matmul, scale u.
            for tb in range(TBS):
                sl = slice(tb * 512, (tb + 1) * 512)
                wb_ps = psauxp.tile([ER, 512], F32, name="wbps", tag="aux")
                nc.tensor.matmul(wb_ps[:], Mm_sb[:], wTb[tb][:], start=True, stop=True)
                nc.scalar.copy(Wb_sb[:, sl], wb_ps[:])
                nc.vector.tensor_tensor(
                    us_sb[:, sl], u_ps[tb][:], Wb_sb[:, sl], ALU.mult,
                )

        # ob=0: experts lag 2 tiles behind their base loops so us (ready only
        # after the chains + w_path) never stalls PE.
        pend = {}
        pend[0] = base_tile(0, 0)
        pend[1] = base_tile(0, 1)
        w_path()
        for ti in range(2, TI):
            pend[ti] = base_tile(0, ti)
            finish_tile(0, ti - 2, pend.pop(ti - 2), veng=(ti % 2 == 0))
        finish_tile(0, TI - 2, pend.pop(TI - 2), veng=False)
        finish_tile(0, TI - 1, pend.pop(TI - 1), veng=True)
        for ob in range(1, OBS):
            for ti in range(TI):
                pss = base_tile(ob, ti)
                finish_tile(ob, ti, pss, veng=(ti % 2 == 0),
                            last=(ob == OBS - 1 and ti == TI - 1))


def build_nc():
    nc = bacc.Bacc("TRN2", target_bir_lowering=False, debug=False, num_devices=N_CORES)
    xh = nc.dram_tensor("xh", [D, NT], BF16, kind="ExternalInput").ap()
    WTo = [
        nc.dram_tensor(f"WTo{ob}", [128, KT, 512], BF16, kind="ExternalInput").ap()
        for ob in range(OBS)
    ]
    AT = nc.dram_tensor("AT", [128, KT, ER], BF16, kind="ExternalInput").ap()
    RT = nc.dram_tensor("RT", [128, KT, 32], BF16, kind="ExternalInput").ap()
    Bc = nc.dram_tensor("Bc", [ER, O], BF16, kind="ExternalInput").ap()
    Mm = nc.dram_tensor("Mm", [E, ER], BF16, kind="ExternalInput").ap()
    out = nc.dram_tensor("out", [NT, O], BF16, kind="ExternalOutput").ap()
    with tile.TileContext(nc) as tc:
        _body(tc, nc, xh, WTo, AT, RT, Bc, Mm, out)
    nc.compile()
    return nc


def get_nc():
    if "nc" not in _NC_CACHE:
        _NC_CACHE["nc"] = build_nc()
    return _NC_CACHE["nc"]


def make_in_maps(x, weight, lora_A, lora_B, router_w):
    x = np.ascontiguousarray(np.asarray(x, dtype=np.float32)).reshape(N_TOK, D)
    weight = np.asarray(weight, dtype=np.float32)
    lora_A = np.asarray(lora_A, dtype=np.float32)
    lora_B = np.asarray(lora_B, dtype=np.float32)
    router_w = np.asarray(router_w, dtype=np.float32)

    def to_pk(a):
        # [D, C] -> [128, KT, C]: partition p holds row k*128+p for each k chunk
        return np.ascontiguousarray(a.reshape(KT, 128, a.shape[1]).transpose(1, 0, 2))

    WT = weight.T.astype(ml_dtypes.bfloat16)  # [D, O]
    # o-block-major contiguous: WTo[ob] = [128, KT, 512] (partition, k-chunk, o)
    WTo = [
        np.ascontiguousarray(
            WT[:, ob * 512:(ob + 1) * 512].reshape(KT, 128, 512).transpose(1, 0, 2)
        )
        for ob in range(OBS)
    ]
    ATm = to_pk(lora_A.reshape(ER, D).T).astype(ml_dtypes.bfloat16)
    # router padded to 32 output cols (zeros) so logits land in a [32, 512]
    # PSUM tile that the 32x32 DVE stream-transpose can consume directly
    Rpad = np.zeros((D, 32), dtype=np.float32)
    Rpad[:, :E] = router_w.T
    RTm = to_pk(Rpad).astype(ml_dtypes.bfloat16)
    Bcm = np.ascontiguousarray(lora_B.transpose(0, 2, 1).reshape(ER, O)).astype(ml_dtypes.bfloat16)
    Mmm = np.zeros((E, ER), dtype=np.float32)
    for e in range(E):
        Mmm[e, e * R:(e + 1) * R] = 1.0
    Mmm = Mmm.astype(ml_dtypes.bfloat16)

    in_maps = []
    for c in range(N_CORES):
        xTc = np.ascontiguousarray(x[c * NT:(c + 1) * NT].T)
        xhc = xTc.astype(ml_dtypes.bfloat16)
        im = {
            "xh": xhc,
            "AT": ATm,
            "RT": RTm,
            "Bc": Bcm,
            "Mm": Mmm,
        }
        for ob in range(OBS):
            im[f"WTo{ob}"] = WTo[ob]
        in_maps.append(im)
    return in_maps


def kernel(x, weight, lora_A, lora_B, router_w):
    global LAST_RESULTS
    from concourse.bass_utils import run_bass_kernel_spmd

    in_maps = make_in_maps(x, weight, lora_A, lora_B, router_w)
    nc = get_nc()
    trace = bool(os.environ.get("KBENCH_TRACE"))
    res = run_bass_kernel_spmd(nc, in_maps, core_ids=list(range(N_CORES)), trace=trace)
    LAST_RESULTS = res
    outs = [np.asarray(res.results[c]["out"], dtype=np.float32) for c in range(N_CORES)]
    return np.concatenate(outs, axis=0).reshape(4, 2048, 2048)
